# revision 14
# baseline (speedup 1.0000x reference)
"""Convex_f forward on 8 trn2 NeuronCores (pure data parallel over batch).

Math: with y = x + param and the interior 3-point stencils
  Dy[i]    = -y[i-1] + 2 y[i] - y[i+1]          (0 at i = 0, N-1)
  mid_y[i] = 0.5 (y[i-1] + y[i+1])
the reference computes out = y - (Dy > 0) * (y - mid_y) - param.
Since y - mid_y = 0.5 * Dy on the interior, this collapses to
  out[i] = x[i] - relu(ctr - 0.5*up - 0.5*dn)   for 0 < i < N-1
  out[i] = x[i]                                  at i = 0, N-1.

The boundary case is folded into the interior formula by padding each
batch with a halo row at both N-ends host-side: x_halo = +1e30 and
param_halo = 0, so y_halo = +1e30 and relu(ctr - 0.5*y_halo - ...) = 0.

Per-core layout: partition p holds J=64 consecutive n-rows (x16 K) per
batch, so the stencil shift is a free-dim offset of K elements and every
DMA is one large transfer with 4KiB+ contiguous runs per partition.

Strategies:
  accum    — load x (HWDGE), copy x->y on ScalarE, land param directly
             into y via SWDGE accum-add DMA. No param tile, no gpsimd
             compute; DVE does the two stencil STTs + final subtract.
  gpsimd_y — load x and param plainly; y = x + param on GpSimd.
"""

import os

import numpy as np

B, N, K = 256, 8192, 16
NCORES = 8
BPC = B // NCORES  # 32 batches per core
P = 128
J = N // P         # 64 n-rows per partition per batch
NP = N + 2         # padded rows per batch
FHB = (J + 2) * K  # 1056 haloed free elems per batch per partition
FIB = J * K        # 1024 interior free elems per batch per partition
BIG = 1.0e30

STRATEGY = os.environ.get("CONVEX_STRATEGY", "bf16")
BPI = int(os.environ.get("CONVEX_BPI", "1"))     # batches per iteration
BUFS = int(os.environ.get("CONVEX_BUFS", "6"))
PIPE = int(os.environ.get("CONVEX_PIPE", "1"))   # sw-pipeline the tail op
WARM = int(os.environ.get("CONVEX_WARM", "0"))   # first iters y-add on DVE

_cache = {}

# Results of the last hardware run (BassKernelResults); test harnesses can
# read exec_time_ns etc. from here after calling kernel().
LAST_RESULTS = None


def _build_nc():
    import concourse.bacc as bacc
    import concourse.bass as bass
    import concourse.mybir as mybir
    from concourse.tile import TileContext

    f32 = mybir.dt.float32
    AO = mybir.AluOpType
    AF = mybir.ActivationFunctionType
    FH = BPI * FHB
    FI = BPI * FIB

    io_dt = mybir.dt.bfloat16 if STRATEGY in ("bf16", "pe_t") else f32
    nc = bacc.Bacc()
    x_d = nc.dram_tensor("x", [BPC, NP, K], io_dt, kind="ExternalInput")
    p_d = nc.dram_tensor("p", [BPC, NP, K], io_dt, kind="ExternalInput")
    o_d = nc.dram_tensor("o", [BPC, N, K], io_dt, kind="ExternalOutput")

    def halo_ap(handle, b0):
        # [p, q, f]: partition p reads padded rows [p*J, p*J + J + 2) of
        # batches b0..b0+BPI-1 (overlapping reads across partitions).
        return bass.AP(handle, b0 * NP * K, [[J * K, P], [NP * K, BPI], [1, FHB]])

    def out_ap(handle, b0):
        return bass.AP(handle, b0 * N * K, [[J * K, P], [N * K, BPI], [1, FIB]])

    n_iter = BPC // BPI
    if STRATEGY == "pe_y":
        return _build_pe_y(nc, bass, mybir, x_d, p_d, o_d, halo_ap, out_ap)
    if STRATEGY == "bf16":
        return _build_bf16(nc, bass, mybir, x_d, p_d, o_d, halo_ap, out_ap)
    if STRATEGY == "pe_t":
        return _build_pe_t(nc, bass, mybir, x_d, p_d, o_d, halo_ap, out_ap)
    with TileContext(nc) as tc:
        with tc.tile_pool(name="io", bufs=BUFS) as pool:
            # stage A state carried to the delayed tail stage
            pend = []

            def stage_a(it):
                b0 = it * BPI
                x_t = pool.tile([P, FH], f32, name="x_t")
                y_t = pool.tile([P, FH], f32, name="y_t")
                d_t = pool.tile([P, FI], f32, name="d_t")
                if STRATEGY == "dve_y2":
                    # 3 tiles only: p lands in y_t, y-add in place, output
                    # in place over x_t's interior view.
                    nc.sync.dma_start(x_t[:], halo_ap(x_d, b0))
                    nc.sync.dma_start(y_t[:], halo_ap(p_d, b0))
                    nc.vector.tensor_tensor(y_t[:], x_t[:], y_t[:], op=AO.add)
                    y3 = y_t.rearrange("p (q f) -> p q f", q=BPI)
                    d3 = d_t.rearrange("p (q f) -> p q f", q=BPI)
                    up = y3[:, :, 0:FIB]
                    ctr = y3[:, :, K:K + FIB]
                    dn = y3[:, :, 2 * K:2 * K + FIB]
                    nc.vector.scalar_tensor_tensor(d3[:], up, -0.5, ctr,
                                                   AO.mult, AO.add)
                    nc.vector.scalar_tensor_tensor(d3[:], dn, -0.5, d3[:],
                                                   AO.mult, AO.add)
                    nc.scalar.activation(d3[:], d3[:], AF.Relu)
                    return (it, x_t, d_t)

                nc.sync.dma_start(x_t[:], halo_ap(x_d, b0))
                if STRATEGY == "accum":
                    # y = x (ScalarE copy), then y += param via SWDGE accum
                    nc.scalar.copy(y_t[:], x_t[:])
                    nc.gpsimd.dma_start(
                        y_t[:], halo_ap(p_d, b0), accum_op=AO.add
                    )
                elif STRATEGY == "dve_y":
                    # y = x + param on DVE, in place over the param tile
                    # (gpsimd elementwise stalls DVE via the shared SBUF
                    # port lock, so gpsimd does no compute at all here)
                    nc.sync.dma_start(y_t[:], halo_ap(p_d, b0))
                    nc.vector.tensor_tensor(y_t[:], x_t[:], y_t[:], op=AO.add)
                else:
                    p_t = pool.tile([P, FH], f32, name="p_t")
                    nc.sync.dma_start(p_t[:], halo_ap(p_d, b0))
                    nc.gpsimd.tensor_tensor(y_t[:], x_t[:], p_t[:], op=AO.add)

                y3 = y_t.rearrange("p (q f) -> p q f", q=BPI)
                d3 = d_t.rearrange("p (q f) -> p q f", q=BPI)
                up = y3[:, :, 0:FIB]
                ctr = y3[:, :, K:K + FIB]
                dn = y3[:, :, 2 * K:2 * K + FIB]

                # e = ctr - 0.5*up ; d = e - 0.5*dn = ctr - 0.5*(up + dn)
                nc.vector.scalar_tensor_tensor(d3[:], up, -0.5, ctr, AO.mult, AO.add)
                nc.vector.scalar_tensor_tensor(d3[:], dn, -0.5, d3[:], AO.mult, AO.add)
                # r = relu(d) in place on ScalarE
                nc.scalar.activation(d3[:], d3[:], AF.Relu)
                return (it, x_t, d_t)

            def stage_b(state):
                it, x_t, d_t = state
                b0 = it * BPI
                x3 = x_t.rearrange("p (q f) -> p q f", q=BPI)
                d3 = d_t.rearrange("p (q f) -> p q f", q=BPI)
                xc = x3[:, :, K:K + FIB]
                if STRATEGY == "dve_y2":
                    # out = x - relu(d), in place over x's interior view
                    nc.vector.tensor_tensor(xc, xc, d3[:], op=AO.subtract)
                    nc.scalar.dma_start(out_ap(o_d, b0), xc)
                    return
                o_t = pool.tile([P, FI], f32, name="o_t")
                o3 = o_t.rearrange("p (q f) -> p q f", q=BPI)
                # out = x - relu(d)
                nc.vector.tensor_tensor(o3[:], xc, d3[:], op=AO.subtract)
                # stores go out on the ACT HWDGE ring so a store waiting on
                # o_t can't head-of-line-block the next loads on the SP ring
                nc.scalar.dma_start(out_ap(o_d, b0), o_t[:])

            for it in range(n_iter):
                pend.append(stage_a(it))
                if len(pend) > PIPE:
                    stage_b(pend.pop(0))
            for s in pend:
                stage_b(s)
    nc.finalize()
    return nc


def _build_pe_t(nc, bass, mybir, x_d, p_d, o_d, halo_ap, out_ap):
    """bf16 I/O + TensorEngine stencil accumulation. DVE's STT ops run at
    half rate (~115 G elem/s) and were the bottleneck of the all-DVE bf16
    build, so the shifted adds go to the (otherwise idle) PE as identity
    matmuls into PSUM:
        ps = x_up + p_up + x_dn + p_dn            (4 matmuls, stationary I)
        u  = 0.5*ps - p_ctr                       (DVE STT)
        o  = min(x_ctr, u)                        (DVE tensor_tensor)
    CONVEX_PE5=1 also folds -2*p_ctr into PSUM via a -2I stationary,
    leaving a single DVE op: o = min(0.5*ps, x_ctr).
    """
    import numpy as np
    import ml_dtypes
    from concourse.tile import TileContext

    bf16 = mybir.dt.bfloat16
    f32 = mybir.dt.float32
    AO = mybir.AluOpType
    FH = BPI * FHB
    FI = BPI * FIB
    n_iter = BPC // BPI
    PE5 = bool(int(os.environ.get("CONVEX_PE5", "0")))
    PSB = int(os.environ.get("CONVEX_PSB", "2"))

    ident_d = nc.inline_tensor(
        np.eye(P, dtype=ml_dtypes.bfloat16), name="ident")
    if PE5:
        neg2_d = nc.inline_tensor(
            (-2.0 * np.eye(P)).astype(ml_dtypes.bfloat16), name="neg2")

    with TileContext(nc) as tc:
        with (
            tc.tile_pool(name="const", bufs=1) as cpool,
            tc.tile_pool(name="io", bufs=BUFS) as pool,
            tc.tile_pool(name="ps", bufs=PSB, space="PSUM") as pspool,
        ):
            # constants go out on the otherwise-idle SWDGE ring so the
            # first x/p loads are at the head of the HWDGE queues
            ident_t = cpool.tile([P, P], bf16, name="ident_t")
            nc.gpsimd.dma_start(ident_t[:], ident_d.ap())
            if PE5:
                neg2_t = cpool.tile([P, P], bf16, name="neg2_t")
                nc.gpsimd.dma_start(neg2_t[:], neg2_d.ap())

            pend = []

            def stage_a(it):
                b0 = it * BPI
                x_t = pool.tile([P, FH], bf16, name="x_t")
                p_t = pool.tile([P, FH], bf16, name="p_t")
                u_t = pool.tile([P, FI], bf16, name="u_t")

                nc.sync.dma_start(x_t[:], halo_ap(x_d, b0))
                nc.scalar.dma_start(p_t[:], halo_ap(p_d, b0))

                x3 = x_t.rearrange("p (q f) -> p q f", q=BPI)
                p3 = p_t.rearrange("p (q f) -> p q f", q=BPI)
                u3 = u_t.rearrange("p (q f) -> p q f", q=BPI)

                for q in range(BPI):
                    ps = pspool.tile([P, FIB], f32, name="ps")
                    for c0 in range(0, FIB, 512):
                        c1 = min(c0 + 512, FIB)
                        cps = ps[:, c0:c1]
                        nc.tensor.matmul(cps, ident_t[:],
                                         x3[:, q, c0:c1],
                                         start=True, stop=False)
                        nc.tensor.matmul(cps, ident_t[:],
                                         x3[:, q, 2 * K + c0:2 * K + c1],
                                         start=False, stop=False)
                        nc.tensor.matmul(cps, ident_t[:],
                                         p3[:, q, c0:c1],
                                         start=False, stop=False)
                        nc.tensor.matmul(cps, ident_t[:],
                                         p3[:, q, 2 * K + c0:2 * K + c1],
                                         start=False, stop=not PE5)
                        if PE5:
                            nc.tensor.matmul(cps, neg2_t[:],
                                             p3[:, q, K + c0:K + c1],
                                             start=False, stop=True)
                    uq = u3[:, q, :]
                    if PE5:
                        # o = min(0.5*ps, x_ctr) — single DVE op
                        nc.vector.scalar_tensor_tensor(
                            uq, ps[:], 0.5, x3[:, q, K:K + FIB],
                            AO.mult, AO.min)
                    else:
                        # u = 0.5*ps - p_ctr
                        nc.vector.scalar_tensor_tensor(
                            uq, ps[:], 0.5, p3[:, q, K:K + FIB],
                            AO.mult, AO.subtract)
                return (it, x_t, u_t)

            def stage_b(state):
                it, x_t, u_t = state
                b0 = it * BPI
                if PE5:
                    nc.gpsimd.dma_start(out_ap(o_d, b0), u_t[:])
                    return
                o_t = pool.tile([P, FI], bf16, name="o_t")
                x3 = x_t.rearrange("p (q f) -> p q f", q=BPI)
                o3 = o_t.rearrange("p (q f) -> p q f", q=BPI)
                u3 = u_t.rearrange("p (q f) -> p q f", q=BPI)
                nc.vector.tensor_tensor(o3[:], x3[:, :, K:K + FIB], u3[:],
                                        op=AO.min)
                nc.gpsimd.dma_start(out_ap(o_d, b0), o_t[:])

            for it in range(n_iter):
                pend.append(stage_a(it))
                if len(pend) > PIPE:
                    stage_b(pend.pop(0))
            for s in pend:
                stage_b(s)
    nc.finalize()
    return nc


def _build_bf16(nc, bass, mybir, x_d, p_d, o_d, halo_ap, out_ap):
    """All-bf16 I/O halves HBM traffic (the kernel is DMA-bound); the
    2e-2 rel-err gate leaves ~30x margin over bf16 rounding. DVE gets 2x
    throughput at 16-bit so all four element-wise passes stay well under
    the DMA floor:
        y  = x + p
        u1 = 0.5*y_up - p_ctr
        u  = 0.5*y_dn + u1       (= x_ctr - d, d the relu argument)
        o  = min(x_ctr, u)       (= x - relu(d))
    Loads split over the SP and ACT HWDGE rings; stores on SWDGE.
    """
    from concourse.tile import TileContext

    bf16 = mybir.dt.bfloat16
    AO = mybir.AluOpType
    FH = BPI * FHB
    FI = BPI * FIB
    n_iter = BPC // BPI

    with TileContext(nc) as tc:
        with tc.tile_pool(name="io", bufs=BUFS) as pool:
            pend = []

            def stage_a(it):
                b0 = it * BPI
                x_t = pool.tile([P, FH], bf16, name="x_t")
                p_t = pool.tile([P, FH], bf16, name="p_t")
                y_t = pool.tile([P, FH], bf16, name="y_t")
                u_t = pool.tile([P, FI], bf16, name="u_t")

                nc.sync.dma_start(x_t[:], halo_ap(x_d, b0))
                nc.scalar.dma_start(p_t[:], halo_ap(p_d, b0))
                nc.vector.tensor_tensor(y_t[:], x_t[:], p_t[:], op=AO.add)

                p3 = p_t.rearrange("p (q f) -> p q f", q=BPI)
                y3 = y_t.rearrange("p (q f) -> p q f", q=BPI)
                u3 = u_t.rearrange("p (q f) -> p q f", q=BPI)
                for q in range(BPI):
                    uq = u3[:, q, :]
                    nc.vector.scalar_tensor_tensor(
                        uq, y3[:, q, 0:FIB], 0.5, p3[:, q, K:K + FIB],
                        AO.mult, AO.subtract)
                    nc.vector.scalar_tensor_tensor(
                        uq, y3[:, q, 2 * K:2 * K + FIB], 0.5, uq,
                        AO.mult, AO.add)
                return (it, x_t, u_t)

            def stage_b(state):
                it, x_t, u_t = state
                b0 = it * BPI
                o_t = pool.tile([P, FI], bf16, name="o_t")
                x3 = x_t.rearrange("p (q f) -> p q f", q=BPI)
                o3 = o_t.rearrange("p (q f) -> p q f", q=BPI)
                u3 = u_t.rearrange("p (q f) -> p q f", q=BPI)
                nc.vector.tensor_tensor(o3[:], x3[:, :, K:K + FIB], u3[:],
                                        op=AO.min)
                nc.gpsimd.dma_start(out_ap(o_d, b0), o_t[:])

            for it in range(n_iter):
                pend.append(stage_a(it))
                if len(pend) > PIPE:
                    stage_b(pend.pop(0))
            for s in pend:
                stage_b(s)
    nc.finalize()
    return nc


def _build_pe_y(nc, bass, mybir, x_d, p_d, o_d, halo_ap, out_ap):
    """y = x + param on the TensorEngine (identity-matmul accumulate into
    PSUM), then per batch on DVE (each op reads at most one PSUM operand):
        u1 = 0.5*y_up - p_ctr
        u  = 0.5*y_dn + u1         (= x_ctr - d, with d the relu argument)
        o  = min(x_ctr, u)         (= x - relu(d))
    No relu, no PSUM->SBUF copy, no y-add on DVE. Loads split over the SP
    and ACT HWDGE rings; stores on SWDGE (GpSimd is otherwise idle).
    """
    import numpy as np
    from concourse.tile import TileContext

    f32 = mybir.dt.float32
    AO = mybir.AluOpType
    FH = BPI * FHB
    FI = BPI * FIB
    n_iter = BPC // BPI

    # bf16 identity is exact (1.0/0.0) and enables fast-weight-load;
    # f32 identity is the proven default
    ident_d = ident_bf_d = None
    if os.environ.get("CONVEX_IDENT_BF16"):
        import ml_dtypes
        ident_bf_d = nc.inline_tensor(
            np.eye(P, dtype=ml_dtypes.bfloat16), name="ident"
        )
    else:
        ident_d = nc.inline_tensor(np.eye(P, dtype=np.float32), name="ident")

    with TileContext(nc) as tc:
        with (
            tc.tile_pool(name="const", bufs=1) as cpool,
            tc.tile_pool(name="io", bufs=BUFS) as pool,
            tc.tile_pool(name="ps", bufs=2, space="PSUM") as pspool,
        ):
            if ident_d is not None:
                ident_t = cpool.tile([P, P], f32, name="ident_t")
                nc.sync.dma_start(ident_t[:], ident_d.ap())
            else:
                ident_t = cpool.tile([P, P], mybir.dt.bfloat16, name="ident_t")
                nc.sync.dma_start(ident_t[:], ident_bf_d.ap())

            pend = []

            def stage_a(it):
                b0 = it * BPI
                x_t = pool.tile([P, FH], f32, name="x_t")
                p_t = pool.tile([P, FH], f32, name="p_t")
                u_t = pool.tile([P, FI], f32, name="u_t")

                nc.sync.dma_start(x_t[:], halo_ap(x_d, b0))
                nc.scalar.dma_start(p_t[:], halo_ap(p_d, b0))

                x3 = x_t.rearrange("p (q f) -> p q f", q=BPI)
                p3 = p_t.rearrange("p (q f) -> p q f", q=BPI)
                u3 = u_t.rearrange("p (q f) -> p q f", q=BPI)

                if it < WARM:
                    # first iterations: y-add on DVE so nothing waits on a
                    # cold TensorEngine chain at startup
                    y_t = pool.tile([P, FH], f32, name="y_t")
                    nc.vector.tensor_tensor(y_t[:], x_t[:], p_t[:], op=AO.add)
                    y3 = y_t.rearrange("p (q f) -> p q f", q=BPI)
                    for q in range(BPI):
                        uq = u3[:, q, :]
                        nc.vector.scalar_tensor_tensor(
                            uq, y3[:, q, 0:FIB], 0.5, p3[:, q, K:K + FIB],
                            AO.mult, AO.subtract)
                        nc.vector.scalar_tensor_tensor(
                            uq, y3[:, q, 2 * K:2 * K + FIB], 0.5, uq,
                            AO.mult, AO.add)
                    return (it, x_t, u_t)

                for q in range(BPI):
                    ps = pspool.tile([P, FHB], f32, name="ps")
                    qo = q * FHB
                    # y = x + p, accumulated on the PE per <=512-col chunk
                    for c0 in range(0, FHB, 512):
                        c1 = min(c0 + 512, FHB)
                        nc.tensor.matmul(ps[:, c0:c1], ident_t[:],
                                         x_t[:, qo + c0:qo + c1],
                                         start=True, stop=False)
                        nc.tensor.matmul(ps[:, c0:c1], ident_t[:],
                                         p_t[:, qo + c0:qo + c1],
                                         start=False, stop=True)
                    uq = u3[:, q, :]
                    # u1 = 0.5*y_up - p_ctr ; u = 0.5*y_dn + u1
                    nc.vector.scalar_tensor_tensor(
                        uq, ps[:, 0:FIB], 0.5, p3[:, q, K:K + FIB],
                        AO.mult, AO.subtract)
                    nc.vector.scalar_tensor_tensor(
                        uq, ps[:, 2 * K:2 * K + FIB], 0.5, uq,
                        AO.mult, AO.add)
                return (it, x_t, u_t)

            def stage_b(state):
                it, x_t, u_t = state
                b0 = it * BPI
                o_t = pool.tile([P, FI], f32, name="o_t")
                x3 = x_t.rearrange("p (q f) -> p q f", q=BPI)
                o3 = o_t.rearrange("p (q f) -> p q f", q=BPI)
                u3 = u_t.rearrange("p (q f) -> p q f", q=BPI)
                # o = min(x_ctr, u) = x - relu(d)
                nc.vector.tensor_tensor(o3[:], x3[:, :, K:K + FIB], u3[:],
                                        op=AO.min)
                nc.gpsimd.dma_start(out_ap(o_d, b0), o_t[:])

            for it in range(n_iter):
                pend.append(stage_a(it))
                if len(pend) > PIPE:
                    stage_b(pend.pop(0))
            for s in pend:
                stage_b(s)
    nc.finalize()
    return nc


def _pad_inputs(x, param):
    # -> per-core padded slabs, shape [NCORES, BPC, NP, K]
    if STRATEGY in ("bf16", "pe_t"):
        import ml_dtypes
        io_np = ml_dtypes.bfloat16
    else:
        io_np = np.float32
    x = np.ascontiguousarray(x, dtype=np.float32).reshape(NCORES, BPC, N, K)
    param = np.ascontiguousarray(param, dtype=np.float32).reshape(NCORES, BPC, N, K)
    xp = np.empty((NCORES, BPC, NP, K), dtype=io_np)
    pp = np.empty((NCORES, BPC, NP, K), dtype=io_np)
    xp[:, :, 1:N + 1] = x.astype(io_np) if io_np is not np.float32 else x
    xp[:, :, 0] = BIG
    xp[:, :, N + 1] = BIG
    pp[:, :, 1:N + 1] = param.astype(io_np) if io_np is not np.float32 else param
    pp[:, :, 0] = 0.0
    pp[:, :, N + 1] = 0.0
    return xp, pp


def kernel(x: np.ndarray, param: np.ndarray) -> np.ndarray:
    global LAST_RESULTS
    from concourse.bass_utils import run_bass_kernel_spmd

    if "nc" not in _cache:
        _cache["nc"] = _build_nc()
    nc = _cache["nc"]

    xp, pp = _pad_inputs(x, param)
    in_maps = [{"x": xp[c], "p": pp[c]} for c in range(NCORES)]

    trace = bool(os.environ.get("BASS_TRACE"))
    res = run_bass_kernel_spmd(
        nc, in_maps, core_ids=list(range(NCORES)), trace=trace
    )
    LAST_RESULTS = res
    out = np.concatenate([res.results[c]["o"] for c in range(NCORES)], axis=0)
    return out.reshape(B, N, K).astype(np.float32, copy=False)



# revision 20
# speedup vs baseline: 1.0053x; 1.0053x over previous
"""Convex_f forward on 8 trn2 NeuronCores (pure data parallel over batch).

Math: with y = x + param and the interior 3-point stencils
  Dy[i]    = -y[i-1] + 2 y[i] - y[i+1]          (0 at i = 0, N-1)
  mid_y[i] = 0.5 (y[i-1] + y[i+1])
the reference computes out = y - (Dy > 0) * (y - mid_y) - param.
Since y - mid_y = 0.5 * Dy on the interior, this collapses to
  out[i] = x[i] - relu(ctr - 0.5*up - 0.5*dn)   for 0 < i < N-1
  out[i] = x[i]                                  at i = 0, N-1.

The boundary case is folded into the interior formula by padding each
batch with a halo row at both N-ends host-side: x_halo = +1e30 and
param_halo = 0, so y_halo = +1e30 and relu(ctr - 0.5*y_halo - ...) = 0.

Per-core layout: partition p holds J=64 consecutive n-rows (x16 K) per
batch, so the stencil shift is a free-dim offset of K elements and every
DMA is one large transfer with 4KiB+ contiguous runs per partition.

Strategies:
  accum    — load x (HWDGE), copy x->y on ScalarE, land param directly
             into y via SWDGE accum-add DMA. No param tile, no gpsimd
             compute; DVE does the two stencil STTs + final subtract.
  gpsimd_y — load x and param plainly; y = x + param on GpSimd.
"""

import os

import numpy as np

B, N, K = 256, 8192, 16
NCORES = 8
BPC = B // NCORES  # 32 batches per core
P = 128
J = N // P         # 64 n-rows per partition per batch
NP = N + 2         # padded rows per batch
FHB = (J + 2) * K  # 1056 haloed free elems per batch per partition
FIB = J * K        # 1024 interior free elems per batch per partition
BIG = 1.0e30

STRATEGY = os.environ.get("CONVEX_STRATEGY", "wstat")
BPI = int(os.environ.get("CONVEX_BPI", "1"))     # batches per iteration
BUFS = int(os.environ.get("CONVEX_BUFS", "6"))
PIPE = int(os.environ.get("CONVEX_PIPE", "1"))   # sw-pipeline the tail op
PSB = int(os.environ.get("CONVEX_PSB", "2"))     # PSUM pool bufs
WARM = int(os.environ.get("CONVEX_WARM", "0"))   # first iters y-add on DVE

_cache = {}

# ---- wstat strategy: n in the partition dim, stencil as one tridiagonal
# matmul per input, p in noise-shaped fp8.
F = BPC * K            # 512 free elems per n-row per core
TSTRIDE = P - 2        # 126 output rows per full tile
_FULL = [TSTRIDE * t for t in range(65)]      # full tiles cover [0, 8190)
WS_LOAD = _FULL + [NP - P]                    # padded-row load starts
WS_R0 = [TSTRIDE * t for t in range(65)] + [8190]   # output-row starts
WS_CNT = [TSTRIDE] * 65 + [2]
WS_PO = [1] * 65 + [8190 - (NP - P) + 1]      # o_t partition offset
WS_NT = len(WS_LOAD)                          # 66 tiles

P8 = os.environ.get("CONVEX_P8", "e3")        # p dtype: e3 | e4 | bf16
O8 = int(os.environ.get("CONVEX_O8", "0"))    # out in fp8e3 too
SHAPE_SWEEPS = int(os.environ.get("CONVEX_SHAPE", "3"))

# Results of the last hardware run (BassKernelResults); test harnesses can
# read exec_time_ns etc. from here after calling kernel().
LAST_RESULTS = None


def _build_nc():
    import concourse.bacc as bacc
    import concourse.bass as bass
    import concourse.mybir as mybir
    from concourse.tile import TileContext

    f32 = mybir.dt.float32
    AO = mybir.AluOpType
    AF = mybir.ActivationFunctionType
    FH = BPI * FHB
    FI = BPI * FIB

    io_dt = mybir.dt.bfloat16 if STRATEGY in ("bf16", "pe_t") else f32
    nc = bacc.Bacc()
    x_d = nc.dram_tensor("x", [BPC, NP, K], io_dt, kind="ExternalInput")
    p_d = nc.dram_tensor("p", [BPC, NP, K], io_dt, kind="ExternalInput")
    o_d = nc.dram_tensor("o", [BPC, N, K], io_dt, kind="ExternalOutput")

    def halo_ap(handle, b0):
        # [p, q, f]: partition p reads padded rows [p*J, p*J + J + 2) of
        # batches b0..b0+BPI-1 (overlapping reads across partitions).
        return bass.AP(handle, b0 * NP * K, [[J * K, P], [NP * K, BPI], [1, FHB]])

    def out_ap(handle, b0):
        return bass.AP(handle, b0 * N * K, [[J * K, P], [N * K, BPI], [1, FIB]])

    n_iter = BPC // BPI
    if STRATEGY == "pe_y":
        return _build_pe_y(nc, bass, mybir, x_d, p_d, o_d, halo_ap, out_ap)
    if STRATEGY == "bf16":
        return _build_bf16(nc, bass, mybir, x_d, p_d, o_d, halo_ap, out_ap)
    if STRATEGY == "pe_t":
        return _build_pe_t(nc, bass, mybir, x_d, p_d, o_d, halo_ap, out_ap)
    with TileContext(nc) as tc:
        with tc.tile_pool(name="io", bufs=BUFS) as pool:
            # stage A state carried to the delayed tail stage
            pend = []

            def stage_a(it):
                b0 = it * BPI
                x_t = pool.tile([P, FH], f32, name="x_t")
                y_t = pool.tile([P, FH], f32, name="y_t")
                d_t = pool.tile([P, FI], f32, name="d_t")
                if STRATEGY == "dve_y2":
                    # 3 tiles only: p lands in y_t, y-add in place, output
                    # in place over x_t's interior view.
                    nc.sync.dma_start(x_t[:], halo_ap(x_d, b0))
                    nc.sync.dma_start(y_t[:], halo_ap(p_d, b0))
                    nc.vector.tensor_tensor(y_t[:], x_t[:], y_t[:], op=AO.add)
                    y3 = y_t.rearrange("p (q f) -> p q f", q=BPI)
                    d3 = d_t.rearrange("p (q f) -> p q f", q=BPI)
                    up = y3[:, :, 0:FIB]
                    ctr = y3[:, :, K:K + FIB]
                    dn = y3[:, :, 2 * K:2 * K + FIB]
                    nc.vector.scalar_tensor_tensor(d3[:], up, -0.5, ctr,
                                                   AO.mult, AO.add)
                    nc.vector.scalar_tensor_tensor(d3[:], dn, -0.5, d3[:],
                                                   AO.mult, AO.add)
                    nc.scalar.activation(d3[:], d3[:], AF.Relu)
                    return (it, x_t, d_t)

                nc.sync.dma_start(x_t[:], halo_ap(x_d, b0))
                if STRATEGY == "accum":
                    # y = x (ScalarE copy), then y += param via SWDGE accum
                    nc.scalar.copy(y_t[:], x_t[:])
                    nc.gpsimd.dma_start(
                        y_t[:], halo_ap(p_d, b0), accum_op=AO.add
                    )
                elif STRATEGY == "dve_y":
                    # y = x + param on DVE, in place over the param tile
                    # (gpsimd elementwise stalls DVE via the shared SBUF
                    # port lock, so gpsimd does no compute at all here)
                    nc.sync.dma_start(y_t[:], halo_ap(p_d, b0))
                    nc.vector.tensor_tensor(y_t[:], x_t[:], y_t[:], op=AO.add)
                else:
                    p_t = pool.tile([P, FH], f32, name="p_t")
                    nc.sync.dma_start(p_t[:], halo_ap(p_d, b0))
                    nc.gpsimd.tensor_tensor(y_t[:], x_t[:], p_t[:], op=AO.add)

                y3 = y_t.rearrange("p (q f) -> p q f", q=BPI)
                d3 = d_t.rearrange("p (q f) -> p q f", q=BPI)
                up = y3[:, :, 0:FIB]
                ctr = y3[:, :, K:K + FIB]
                dn = y3[:, :, 2 * K:2 * K + FIB]

                # e = ctr - 0.5*up ; d = e - 0.5*dn = ctr - 0.5*(up + dn)
                nc.vector.scalar_tensor_tensor(d3[:], up, -0.5, ctr, AO.mult, AO.add)
                nc.vector.scalar_tensor_tensor(d3[:], dn, -0.5, d3[:], AO.mult, AO.add)
                # r = relu(d) in place on ScalarE
                nc.scalar.activation(d3[:], d3[:], AF.Relu)
                return (it, x_t, d_t)

            def stage_b(state):
                it, x_t, d_t = state
                b0 = it * BPI
                x3 = x_t.rearrange("p (q f) -> p q f", q=BPI)
                d3 = d_t.rearrange("p (q f) -> p q f", q=BPI)
                xc = x3[:, :, K:K + FIB]
                if STRATEGY == "dve_y2":
                    # out = x - relu(d), in place over x's interior view
                    nc.vector.tensor_tensor(xc, xc, d3[:], op=AO.subtract)
                    nc.scalar.dma_start(out_ap(o_d, b0), xc)
                    return
                o_t = pool.tile([P, FI], f32, name="o_t")
                o3 = o_t.rearrange("p (q f) -> p q f", q=BPI)
                # out = x - relu(d)
                nc.vector.tensor_tensor(o3[:], xc, d3[:], op=AO.subtract)
                # stores go out on the ACT HWDGE ring so a store waiting on
                # o_t can't head-of-line-block the next loads on the SP ring
                nc.scalar.dma_start(out_ap(o_d, b0), o_t[:])

            for it in range(n_iter):
                pend.append(stage_a(it))
                if len(pend) > PIPE:
                    stage_b(pend.pop(0))
            for s in pend:
                stage_b(s)
    nc.finalize()
    return nc


def _np_dt(mybir, dt):
    return mybir.dt.np(dt)


def _build_wstat():
    """Layout v3: partition dim = n (128 consecutive padded rows per tile,
    tiles overlap by 2), free dim = (batch, K) = 512. The whole 3-point
    stencil becomes ONE tridiagonal stationary W = (0.5, -1, 0.5):
        ps  = W @ x_tile + W @ p_tile          (2 matmuls, PSUM f32)
        o   = min(ps, 0) + x_tile              (single DVE STT)
    p rides in fp8 (only the PE reads it); x/o in bf16. Boundary rows
    use the BIG-sentinel x pad so out = x there.
    """
    import numpy as np
    import ml_dtypes
    import concourse.bacc as bacc
    import concourse.bass as bass
    import concourse.mybir as mybir
    from concourse.tile import TileContext

    f32 = mybir.dt.float32
    bf16 = mybir.dt.bfloat16
    AO = mybir.AluOpType
    p_dt = {"e3": mybir.dt.float8e3, "e4": mybir.dt.float8e4,
            "bf16": bf16}[P8]
    o_dt = mybir.dt.float8e3 if O8 else bf16
    p_np = _np_dt(mybir, p_dt)

    nc = bacc.Bacc()
    x_d = nc.dram_tensor("x", [NP, F], bf16, kind="ExternalInput")
    p_d = nc.dram_tensor("p", [NP, F], p_dt, kind="ExternalInput")
    o_d = nc.dram_tensor("o", [N, F], o_dt, kind="ExternalOutput")

    tri = (0.5 * np.eye(P, k=1) + 0.5 * np.eye(P, k=-1) - np.eye(P))
    wx_d = nc.inline_tensor(tri.astype(ml_dtypes.bfloat16), name="wx")
    wp_d = nc.inline_tensor(tri.astype(p_np), name="wp")

    with TileContext(nc) as tc:
        with (
            tc.tile_pool(name="const", bufs=1) as cpool,
            tc.tile_pool(name="io", bufs=BUFS) as pool,
            tc.tile_pool(name="ps", bufs=PSB, space="PSUM") as pspool,
        ):
            wx_t = cpool.tile([P, P], bf16, name="wx_t")
            wp_t = cpool.tile([P, P], p_dt, name="wp_t")
            nc.gpsimd.dma_start(wx_t[:], wx_d.ap())
            nc.gpsimd.dma_start(wp_t[:], wp_d.ap())

            pend = []

            def stage_a(t):
                s = WS_LOAD[t]
                x_t = pool.tile([P, F], bf16, name="x_t")
                p_t = pool.tile([P, F], p_dt, name="p_t")
                o_t = pool.tile([P, F], o_dt, name="o_t")
                nc.sync.dma_start(
                    x_t[:], bass.AP(x_d, s * F, [[F, P], [1, F]]))
                nc.scalar.dma_start(
                    p_t[:], bass.AP(p_d, s * F, [[F, P], [1, F]]))
                ps = pspool.tile([P, F], f32, name="ps")
                nc.tensor.matmul(ps[:], wx_t[:], x_t[:],
                                 start=True, stop=False)
                nc.tensor.matmul(ps[:], wp_t[:], p_t[:],
                                 start=False, stop=True)
                # o = min(w, 0) + x
                nc.vector.scalar_tensor_tensor(
                    o_t[:], ps[:], 0.0, x_t[:], AO.min, AO.add)
                return (t, o_t)

            def stage_b(state):
                t, o_t = state
                po, r0, cnt = WS_PO[t], WS_R0[t], WS_CNT[t]
                nc.gpsimd.dma_start(
                    bass.AP(o_d, r0 * F, [[F, cnt], [1, F]]),
                    o_t[po:po + cnt, :])

            for t in range(WS_NT):
                pend.append(stage_a(t))
                if len(pend) > PIPE:
                    stage_b(pend.pop(0))
            for s_ in pend:
                stage_b(s_)
    nc.finalize()
    return nc


def _shape_quant(p, f8):
    """Noise-shape p's quantization error toward low frequencies along n
    (the stencil (0.5,-1,0.5) is a high-pass and kills them): red-black
    coordinate descent on || h * (q - p) ||^2 over the fp8 grid."""
    import numpy as np

    p = np.ascontiguousarray(p, dtype=np.float32)
    q = p.astype(f8).astype(np.float32)
    n = p.shape[1]
    for _ in range(SHAPE_SWEEPS):
        for par in (0, 1):
            d = q - p
            dm2 = np.zeros_like(d); dm1 = np.zeros_like(d)
            dp1 = np.zeros_like(d); dp2 = np.zeros_like(d)
            dm2[:, 2:] = d[:, :-2]; dm1[:, 1:] = d[:, :-1]
            dp1[:, :-1] = d[:, 1:]; dp2[:, :-2] = d[:, 2:]
            a = 0.5 * dm2 - dm1
            b = 0.5 * (dm1 + dp1)
            c = 0.5 * dp2 - dp1
            dstar = (b - 0.5 * a - 0.5 * c) * (1.0 / 1.5)
            qn = (p + dstar).astype(f8).astype(np.float32)
            q[:, par::2] = qn[:, par::2]
    return q.astype(f8)


def _marshal_wstat(x, param):
    import numpy as np
    import ml_dtypes
    import concourse.mybir as mybir

    bf = ml_dtypes.bfloat16
    p_np = {"e3": ml_dtypes.float8_e3m4, "e4": ml_dtypes.float8_e4m3,
            "bf16": bf}[P8]

    x = np.ascontiguousarray(x, dtype=np.float32).reshape(NCORES, BPC, N, K)
    param = np.ascontiguousarray(param, dtype=np.float32)
    if P8 == "bf16":
        q = param.astype(bf).astype(p_np)
    else:
        q = _shape_quant(param.reshape(B, N, K), p_np)
    q = q.reshape(NCORES, BPC, N, K)

    in_maps = []
    for c in range(NCORES):
        xp = np.empty((NP, BPC, K), dtype=bf)
        xp[0] = BIG
        xp[NP - 1] = BIG
        xp[1:NP - 1] = x[c].transpose(1, 0, 2).astype(bf)
        pp = np.zeros((NP, BPC, K), dtype=p_np)
        pp[1:NP - 1] = q[c].transpose(1, 0, 2)
        in_maps.append({"x": xp.reshape(NP, F), "p": pp.reshape(NP, F)})
    return in_maps


def _build_pe_t(nc, bass, mybir, x_d, p_d, o_d, halo_ap, out_ap):
    """bf16 I/O + TensorEngine stencil accumulation. DVE's STT ops run at
    half rate (~115 G elem/s) and were the bottleneck of the all-DVE bf16
    build, so the shifted adds go to the (otherwise idle) PE as identity
    matmuls into PSUM:
        ps = x_up + p_up + x_dn + p_dn            (4 matmuls, stationary I)
        u  = 0.5*ps - p_ctr                       (DVE STT)
        o  = min(x_ctr, u)                        (DVE tensor_tensor)
    CONVEX_PE5=1 also folds -2*p_ctr into PSUM via a -2I stationary,
    leaving a single DVE op: o = min(0.5*ps, x_ctr).
    """
    import numpy as np
    import ml_dtypes
    from concourse.tile import TileContext

    bf16 = mybir.dt.bfloat16
    f32 = mybir.dt.float32
    AO = mybir.AluOpType
    FH = BPI * FHB
    FI = BPI * FIB
    n_iter = BPC // BPI
    PE5 = bool(int(os.environ.get("CONVEX_PE5", "0")))

    ident_d = nc.inline_tensor(
        np.eye(P, dtype=ml_dtypes.bfloat16), name="ident")
    if PE5:
        neg2_d = nc.inline_tensor(
            (-2.0 * np.eye(P)).astype(ml_dtypes.bfloat16), name="neg2")

    with TileContext(nc) as tc:
        with (
            tc.tile_pool(name="const", bufs=1) as cpool,
            tc.tile_pool(name="io", bufs=BUFS) as pool,
            tc.tile_pool(name="ps", bufs=PSB, space="PSUM") as pspool,
        ):
            # constants go out on the otherwise-idle SWDGE ring so the
            # first x/p loads are at the head of the HWDGE queues
            ident_t = cpool.tile([P, P], bf16, name="ident_t")
            nc.gpsimd.dma_start(ident_t[:], ident_d.ap())
            if PE5:
                neg2_t = cpool.tile([P, P], bf16, name="neg2_t")
                nc.gpsimd.dma_start(neg2_t[:], neg2_d.ap())

            pend = []

            def stage_a(it):
                b0 = it * BPI
                x_t = pool.tile([P, FH], bf16, name="x_t")
                p_t = pool.tile([P, FH], bf16, name="p_t")
                u_t = pool.tile([P, FI], bf16, name="u_t")

                nc.sync.dma_start(x_t[:], halo_ap(x_d, b0))
                nc.scalar.dma_start(p_t[:], halo_ap(p_d, b0))

                x3 = x_t.rearrange("p (q f) -> p q f", q=BPI)
                p3 = p_t.rearrange("p (q f) -> p q f", q=BPI)
                u3 = u_t.rearrange("p (q f) -> p q f", q=BPI)

                for q in range(BPI):
                    ps = pspool.tile([P, FIB], f32, name="ps")
                    for c0 in range(0, FIB, 512):
                        c1 = min(c0 + 512, FIB)
                        cps = ps[:, c0:c1]
                        nc.tensor.matmul(cps, ident_t[:],
                                         x3[:, q, c0:c1],
                                         start=True, stop=False)
                        nc.tensor.matmul(cps, ident_t[:],
                                         x3[:, q, 2 * K + c0:2 * K + c1],
                                         start=False, stop=False)
                        nc.tensor.matmul(cps, ident_t[:],
                                         p3[:, q, c0:c1],
                                         start=False, stop=False)
                        nc.tensor.matmul(cps, ident_t[:],
                                         p3[:, q, 2 * K + c0:2 * K + c1],
                                         start=False, stop=not PE5)
                        if PE5:
                            nc.tensor.matmul(cps, neg2_t[:],
                                             p3[:, q, K + c0:K + c1],
                                             start=False, stop=True)
                    uq = u3[:, q, :]
                    if PE5:
                        # o = min(0.5*ps, x_ctr) — single DVE op
                        nc.vector.scalar_tensor_tensor(
                            uq, ps[:], 0.5, x3[:, q, K:K + FIB],
                            AO.mult, AO.min)
                    else:
                        # u = 0.5*ps - p_ctr
                        nc.vector.scalar_tensor_tensor(
                            uq, ps[:], 0.5, p3[:, q, K:K + FIB],
                            AO.mult, AO.subtract)
                return (it, x_t, u_t)

            def stage_b(state):
                it, x_t, u_t = state
                b0 = it * BPI
                if PE5:
                    nc.gpsimd.dma_start(out_ap(o_d, b0), u_t[:])
                    return
                o_t = pool.tile([P, FI], bf16, name="o_t")
                x3 = x_t.rearrange("p (q f) -> p q f", q=BPI)
                o3 = o_t.rearrange("p (q f) -> p q f", q=BPI)
                u3 = u_t.rearrange("p (q f) -> p q f", q=BPI)
                nc.vector.tensor_tensor(o3[:], x3[:, :, K:K + FIB], u3[:],
                                        op=AO.min)
                nc.gpsimd.dma_start(out_ap(o_d, b0), o_t[:])

            for it in range(n_iter):
                pend.append(stage_a(it))
                if len(pend) > PIPE:
                    stage_b(pend.pop(0))
            for s in pend:
                stage_b(s)
    nc.finalize()
    return nc


def _build_bf16(nc, bass, mybir, x_d, p_d, o_d, halo_ap, out_ap):
    """All-bf16 I/O halves HBM traffic (the kernel is DMA-bound); the
    2e-2 rel-err gate leaves ~30x margin over bf16 rounding. DVE gets 2x
    throughput at 16-bit so all four element-wise passes stay well under
    the DMA floor:
        y  = x + p
        u1 = 0.5*y_up - p_ctr
        u  = 0.5*y_dn + u1       (= x_ctr - d, d the relu argument)
        o  = min(x_ctr, u)       (= x - relu(d))
    Loads split over the SP and ACT HWDGE rings; stores on SWDGE.
    """
    from concourse.tile import TileContext

    bf16 = mybir.dt.bfloat16
    AO = mybir.AluOpType
    FH = BPI * FHB
    FI = BPI * FIB
    n_iter = BPC // BPI

    with TileContext(nc) as tc:
        with tc.tile_pool(name="io", bufs=BUFS) as pool:
            pend = []

            def stage_a(it):
                b0 = it * BPI
                x_t = pool.tile([P, FH], bf16, name="x_t")
                p_t = pool.tile([P, FH], bf16, name="p_t")
                y_t = pool.tile([P, FH], bf16, name="y_t")
                u_t = pool.tile([P, FI], bf16, name="u_t")

                nc.sync.dma_start(x_t[:], halo_ap(x_d, b0))
                nc.scalar.dma_start(p_t[:], halo_ap(p_d, b0))
                nc.vector.tensor_tensor(y_t[:], x_t[:], p_t[:], op=AO.add)

                p3 = p_t.rearrange("p (q f) -> p q f", q=BPI)
                y3 = y_t.rearrange("p (q f) -> p q f", q=BPI)
                u3 = u_t.rearrange("p (q f) -> p q f", q=BPI)
                for q in range(BPI):
                    uq = u3[:, q, :]
                    nc.vector.scalar_tensor_tensor(
                        uq, y3[:, q, 0:FIB], 0.5, p3[:, q, K:K + FIB],
                        AO.mult, AO.subtract)
                    nc.vector.scalar_tensor_tensor(
                        uq, y3[:, q, 2 * K:2 * K + FIB], 0.5, uq,
                        AO.mult, AO.add)
                return (it, x_t, u_t)

            def stage_b(state):
                it, x_t, u_t = state
                b0 = it * BPI
                o_t = pool.tile([P, FI], bf16, name="o_t")
                x3 = x_t.rearrange("p (q f) -> p q f", q=BPI)
                o3 = o_t.rearrange("p (q f) -> p q f", q=BPI)
                u3 = u_t.rearrange("p (q f) -> p q f", q=BPI)
                nc.vector.tensor_tensor(o3[:], x3[:, :, K:K + FIB], u3[:],
                                        op=AO.min)
                nc.gpsimd.dma_start(out_ap(o_d, b0), o_t[:])

            for it in range(n_iter):
                pend.append(stage_a(it))
                if len(pend) > PIPE:
                    stage_b(pend.pop(0))
            for s in pend:
                stage_b(s)
    nc.finalize()
    return nc


def _build_pe_y(nc, bass, mybir, x_d, p_d, o_d, halo_ap, out_ap):
    """y = x + param on the TensorEngine (identity-matmul accumulate into
    PSUM), then per batch on DVE (each op reads at most one PSUM operand):
        u1 = 0.5*y_up - p_ctr
        u  = 0.5*y_dn + u1         (= x_ctr - d, with d the relu argument)
        o  = min(x_ctr, u)         (= x - relu(d))
    No relu, no PSUM->SBUF copy, no y-add on DVE. Loads split over the SP
    and ACT HWDGE rings; stores on SWDGE (GpSimd is otherwise idle).
    """
    import numpy as np
    from concourse.tile import TileContext

    f32 = mybir.dt.float32
    AO = mybir.AluOpType
    FH = BPI * FHB
    FI = BPI * FIB
    n_iter = BPC // BPI

    # bf16 identity is exact (1.0/0.0) and enables fast-weight-load;
    # f32 identity is the proven default
    ident_d = ident_bf_d = None
    if os.environ.get("CONVEX_IDENT_BF16"):
        import ml_dtypes
        ident_bf_d = nc.inline_tensor(
            np.eye(P, dtype=ml_dtypes.bfloat16), name="ident"
        )
    else:
        ident_d = nc.inline_tensor(np.eye(P, dtype=np.float32), name="ident")

    with TileContext(nc) as tc:
        with (
            tc.tile_pool(name="const", bufs=1) as cpool,
            tc.tile_pool(name="io", bufs=BUFS) as pool,
            tc.tile_pool(name="ps", bufs=2, space="PSUM") as pspool,
        ):
            if ident_d is not None:
                ident_t = cpool.tile([P, P], f32, name="ident_t")
                nc.sync.dma_start(ident_t[:], ident_d.ap())
            else:
                ident_t = cpool.tile([P, P], mybir.dt.bfloat16, name="ident_t")
                nc.sync.dma_start(ident_t[:], ident_bf_d.ap())

            pend = []

            def stage_a(it):
                b0 = it * BPI
                x_t = pool.tile([P, FH], f32, name="x_t")
                p_t = pool.tile([P, FH], f32, name="p_t")
                u_t = pool.tile([P, FI], f32, name="u_t")

                nc.sync.dma_start(x_t[:], halo_ap(x_d, b0))
                nc.scalar.dma_start(p_t[:], halo_ap(p_d, b0))

                x3 = x_t.rearrange("p (q f) -> p q f", q=BPI)
                p3 = p_t.rearrange("p (q f) -> p q f", q=BPI)
                u3 = u_t.rearrange("p (q f) -> p q f", q=BPI)

                if it < WARM:
                    # first iterations: y-add on DVE so nothing waits on a
                    # cold TensorEngine chain at startup
                    y_t = pool.tile([P, FH], f32, name="y_t")
                    nc.vector.tensor_tensor(y_t[:], x_t[:], p_t[:], op=AO.add)
                    y3 = y_t.rearrange("p (q f) -> p q f", q=BPI)
                    for q in range(BPI):
                        uq = u3[:, q, :]
                        nc.vector.scalar_tensor_tensor(
                            uq, y3[:, q, 0:FIB], 0.5, p3[:, q, K:K + FIB],
                            AO.mult, AO.subtract)
                        nc.vector.scalar_tensor_tensor(
                            uq, y3[:, q, 2 * K:2 * K + FIB], 0.5, uq,
                            AO.mult, AO.add)
                    return (it, x_t, u_t)

                for q in range(BPI):
                    ps = pspool.tile([P, FHB], f32, name="ps")
                    qo = q * FHB
                    # y = x + p, accumulated on the PE per <=512-col chunk
                    for c0 in range(0, FHB, 512):
                        c1 = min(c0 + 512, FHB)
                        nc.tensor.matmul(ps[:, c0:c1], ident_t[:],
                                         x_t[:, qo + c0:qo + c1],
                                         start=True, stop=False)
                        nc.tensor.matmul(ps[:, c0:c1], ident_t[:],
                                         p_t[:, qo + c0:qo + c1],
                                         start=False, stop=True)
                    uq = u3[:, q, :]
                    # u1 = 0.5*y_up - p_ctr ; u = 0.5*y_dn + u1
                    nc.vector.scalar_tensor_tensor(
                        uq, ps[:, 0:FIB], 0.5, p3[:, q, K:K + FIB],
                        AO.mult, AO.subtract)
                    nc.vector.scalar_tensor_tensor(
                        uq, ps[:, 2 * K:2 * K + FIB], 0.5, uq,
                        AO.mult, AO.add)
                return (it, x_t, u_t)

            def stage_b(state):
                it, x_t, u_t = state
                b0 = it * BPI
                o_t = pool.tile([P, FI], f32, name="o_t")
                x3 = x_t.rearrange("p (q f) -> p q f", q=BPI)
                o3 = o_t.rearrange("p (q f) -> p q f", q=BPI)
                u3 = u_t.rearrange("p (q f) -> p q f", q=BPI)
                # o = min(x_ctr, u) = x - relu(d)
                nc.vector.tensor_tensor(o3[:], x3[:, :, K:K + FIB], u3[:],
                                        op=AO.min)
                nc.gpsimd.dma_start(out_ap(o_d, b0), o_t[:])

            for it in range(n_iter):
                pend.append(stage_a(it))
                if len(pend) > PIPE:
                    stage_b(pend.pop(0))
            for s in pend:
                stage_b(s)
    nc.finalize()
    return nc


def _pad_inputs(x, param):
    # -> per-core padded slabs, shape [NCORES, BPC, NP, K]
    if STRATEGY in ("bf16", "pe_t"):
        import ml_dtypes
        io_np = ml_dtypes.bfloat16
    else:
        io_np = np.float32
    x = np.ascontiguousarray(x, dtype=np.float32).reshape(NCORES, BPC, N, K)
    param = np.ascontiguousarray(param, dtype=np.float32).reshape(NCORES, BPC, N, K)
    xp = np.empty((NCORES, BPC, NP, K), dtype=io_np)
    pp = np.empty((NCORES, BPC, NP, K), dtype=io_np)
    xp[:, :, 1:N + 1] = x.astype(io_np) if io_np is not np.float32 else x
    xp[:, :, 0] = BIG
    xp[:, :, N + 1] = BIG
    pp[:, :, 1:N + 1] = param.astype(io_np) if io_np is not np.float32 else param
    pp[:, :, 0] = 0.0
    pp[:, :, N + 1] = 0.0
    return xp, pp


def kernel(x: np.ndarray, param: np.ndarray) -> np.ndarray:
    global LAST_RESULTS
    from concourse.bass_utils import run_bass_kernel_spmd

    if "nc" not in _cache:
        _cache["nc"] = _build_wstat() if STRATEGY == "wstat" else _build_nc()
    nc = _cache["nc"]

    if STRATEGY == "wstat":
        in_maps = _marshal_wstat(x, param)
    else:
        xp, pp = _pad_inputs(x, param)
        in_maps = [{"x": xp[c], "p": pp[c]} for c in range(NCORES)]

    trace = bool(os.environ.get("BASS_TRACE"))
    res = run_bass_kernel_spmd(
        nc, in_maps, core_ids=list(range(NCORES)), trace=trace
    )
    LAST_RESULTS = res
    if STRATEGY == "wstat":
        out = np.stack([
            np.asarray(res.results[c]["o"])
            .astype(np.float32)
            .reshape(N, BPC, K)
            .transpose(1, 0, 2)
            for c in range(NCORES)
        ])
        return np.ascontiguousarray(out).reshape(B, N, K)
    out = np.concatenate([res.results[c]["o"] for c in range(NCORES)], axis=0)
    return out.reshape(B, N, K).astype(np.float32, copy=False)



# revision 22
# speedup vs baseline: 1.4256x; 1.4182x over previous
"""Convex_f forward on 8 trn2 NeuronCores (pure data parallel over batch).

Math: with y = x + param and the interior 3-point stencils
  Dy[i]    = -y[i-1] + 2 y[i] - y[i+1]          (0 at i = 0, N-1)
  mid_y[i] = 0.5 (y[i-1] + y[i+1])
the reference computes out = y - (Dy > 0) * (y - mid_y) - param.
Since y - mid_y = 0.5 * Dy on the interior, this collapses to
  out[i] = x[i] - relu(ctr - 0.5*up - 0.5*dn)   for 0 < i < N-1
  out[i] = x[i]                                  at i = 0, N-1.

The boundary case is folded into the interior formula by padding each
batch with a halo row at both N-ends host-side: x_halo = +1e30 and
param_halo = 0, so y_halo = +1e30 and relu(ctr - 0.5*y_halo - ...) = 0.

Per-core layout: partition p holds J=64 consecutive n-rows (x16 K) per
batch, so the stencil shift is a free-dim offset of K elements and every
DMA is one large transfer with 4KiB+ contiguous runs per partition.

Strategies:
  accum    — load x (HWDGE), copy x->y on ScalarE, land param directly
             into y via SWDGE accum-add DMA. No param tile, no gpsimd
             compute; DVE does the two stencil STTs + final subtract.
  gpsimd_y — load x and param plainly; y = x + param on GpSimd.
"""

import os

import numpy as np

B, N, K = 256, 8192, 16
NCORES = 8
BPC = B // NCORES  # 32 batches per core
P = 128
J = N // P         # 64 n-rows per partition per batch
NP = N + 2         # padded rows per batch
FHB = (J + 2) * K  # 1056 haloed free elems per batch per partition
FIB = J * K        # 1024 interior free elems per batch per partition
BIG = 1.0e30

STRATEGY = os.environ.get("CONVEX_STRATEGY", "wstat")
BPI = int(os.environ.get("CONVEX_BPI", "1"))     # batches per iteration
BUFS = int(os.environ.get("CONVEX_BUFS", "6"))
PIPE = int(os.environ.get("CONVEX_PIPE", "1"))   # sw-pipeline the tail op
PSB = int(os.environ.get("CONVEX_PSB", "2"))     # PSUM pool bufs
WARM = int(os.environ.get("CONVEX_WARM", "0"))   # first iters y-add on DVE

_cache = {}

# ---- wstat strategy: n in the partition dim, stencil as one tridiagonal
# matmul per input, p in noise-shaped fp8.
F = BPC * K            # 512 free elems per n-row per core
TSTRIDE = P - 2        # 126 output rows per full tile
_FULL = [TSTRIDE * t for t in range(65)]      # full tiles cover [0, 8190)
WS_LOAD = _FULL + [NP - P]                    # padded-row load starts
WS_R0 = [TSTRIDE * t for t in range(65)] + [8190]   # output-row starts
WS_CNT = [TSTRIDE] * 65 + [2]
WS_PO = [1] * 65 + [8190 - (NP - P) + 1]      # o_t partition offset
WS_NT = len(WS_LOAD)                          # 66 tiles

P8 = os.environ.get("CONVEX_P8", "e3")        # p dtype: e3 | e4 | bf16
O8 = int(os.environ.get("CONVEX_O8", "0"))    # out in fp8e3 too
SHAPE_SWEEPS = int(os.environ.get("CONVEX_SHAPE", "3"))
WT = int(os.environ.get("CONVEX_WT", "2"))    # full tiles per iteration

# Results of the last hardware run (BassKernelResults); test harnesses can
# read exec_time_ns etc. from here after calling kernel().
LAST_RESULTS = None


def _build_nc():
    import concourse.bacc as bacc
    import concourse.bass as bass
    import concourse.mybir as mybir
    from concourse.tile import TileContext

    f32 = mybir.dt.float32
    AO = mybir.AluOpType
    AF = mybir.ActivationFunctionType
    FH = BPI * FHB
    FI = BPI * FIB

    io_dt = mybir.dt.bfloat16 if STRATEGY in ("bf16", "pe_t") else f32
    nc = bacc.Bacc()
    x_d = nc.dram_tensor("x", [BPC, NP, K], io_dt, kind="ExternalInput")
    p_d = nc.dram_tensor("p", [BPC, NP, K], io_dt, kind="ExternalInput")
    o_d = nc.dram_tensor("o", [BPC, N, K], io_dt, kind="ExternalOutput")

    def halo_ap(handle, b0):
        # [p, q, f]: partition p reads padded rows [p*J, p*J + J + 2) of
        # batches b0..b0+BPI-1 (overlapping reads across partitions).
        return bass.AP(handle, b0 * NP * K, [[J * K, P], [NP * K, BPI], [1, FHB]])

    def out_ap(handle, b0):
        return bass.AP(handle, b0 * N * K, [[J * K, P], [N * K, BPI], [1, FIB]])

    n_iter = BPC // BPI
    if STRATEGY == "pe_y":
        return _build_pe_y(nc, bass, mybir, x_d, p_d, o_d, halo_ap, out_ap)
    if STRATEGY == "bf16":
        return _build_bf16(nc, bass, mybir, x_d, p_d, o_d, halo_ap, out_ap)
    if STRATEGY == "pe_t":
        return _build_pe_t(nc, bass, mybir, x_d, p_d, o_d, halo_ap, out_ap)
    with TileContext(nc) as tc:
        with tc.tile_pool(name="io", bufs=BUFS) as pool:
            # stage A state carried to the delayed tail stage
            pend = []

            def stage_a(it):
                b0 = it * BPI
                x_t = pool.tile([P, FH], f32, name="x_t")
                y_t = pool.tile([P, FH], f32, name="y_t")
                d_t = pool.tile([P, FI], f32, name="d_t")
                if STRATEGY == "dve_y2":
                    # 3 tiles only: p lands in y_t, y-add in place, output
                    # in place over x_t's interior view.
                    nc.sync.dma_start(x_t[:], halo_ap(x_d, b0))
                    nc.sync.dma_start(y_t[:], halo_ap(p_d, b0))
                    nc.vector.tensor_tensor(y_t[:], x_t[:], y_t[:], op=AO.add)
                    y3 = y_t.rearrange("p (q f) -> p q f", q=BPI)
                    d3 = d_t.rearrange("p (q f) -> p q f", q=BPI)
                    up = y3[:, :, 0:FIB]
                    ctr = y3[:, :, K:K + FIB]
                    dn = y3[:, :, 2 * K:2 * K + FIB]
                    nc.vector.scalar_tensor_tensor(d3[:], up, -0.5, ctr,
                                                   AO.mult, AO.add)
                    nc.vector.scalar_tensor_tensor(d3[:], dn, -0.5, d3[:],
                                                   AO.mult, AO.add)
                    nc.scalar.activation(d3[:], d3[:], AF.Relu)
                    return (it, x_t, d_t)

                nc.sync.dma_start(x_t[:], halo_ap(x_d, b0))
                if STRATEGY == "accum":
                    # y = x (ScalarE copy), then y += param via SWDGE accum
                    nc.scalar.copy(y_t[:], x_t[:])
                    nc.gpsimd.dma_start(
                        y_t[:], halo_ap(p_d, b0), accum_op=AO.add
                    )
                elif STRATEGY == "dve_y":
                    # y = x + param on DVE, in place over the param tile
                    # (gpsimd elementwise stalls DVE via the shared SBUF
                    # port lock, so gpsimd does no compute at all here)
                    nc.sync.dma_start(y_t[:], halo_ap(p_d, b0))
                    nc.vector.tensor_tensor(y_t[:], x_t[:], y_t[:], op=AO.add)
                else:
                    p_t = pool.tile([P, FH], f32, name="p_t")
                    nc.sync.dma_start(p_t[:], halo_ap(p_d, b0))
                    nc.gpsimd.tensor_tensor(y_t[:], x_t[:], p_t[:], op=AO.add)

                y3 = y_t.rearrange("p (q f) -> p q f", q=BPI)
                d3 = d_t.rearrange("p (q f) -> p q f", q=BPI)
                up = y3[:, :, 0:FIB]
                ctr = y3[:, :, K:K + FIB]
                dn = y3[:, :, 2 * K:2 * K + FIB]

                # e = ctr - 0.5*up ; d = e - 0.5*dn = ctr - 0.5*(up + dn)
                nc.vector.scalar_tensor_tensor(d3[:], up, -0.5, ctr, AO.mult, AO.add)
                nc.vector.scalar_tensor_tensor(d3[:], dn, -0.5, d3[:], AO.mult, AO.add)
                # r = relu(d) in place on ScalarE
                nc.scalar.activation(d3[:], d3[:], AF.Relu)
                return (it, x_t, d_t)

            def stage_b(state):
                it, x_t, d_t = state
                b0 = it * BPI
                x3 = x_t.rearrange("p (q f) -> p q f", q=BPI)
                d3 = d_t.rearrange("p (q f) -> p q f", q=BPI)
                xc = x3[:, :, K:K + FIB]
                if STRATEGY == "dve_y2":
                    # out = x - relu(d), in place over x's interior view
                    nc.vector.tensor_tensor(xc, xc, d3[:], op=AO.subtract)
                    nc.scalar.dma_start(out_ap(o_d, b0), xc)
                    return
                o_t = pool.tile([P, FI], f32, name="o_t")
                o3 = o_t.rearrange("p (q f) -> p q f", q=BPI)
                # out = x - relu(d)
                nc.vector.tensor_tensor(o3[:], xc, d3[:], op=AO.subtract)
                # stores go out on the ACT HWDGE ring so a store waiting on
                # o_t can't head-of-line-block the next loads on the SP ring
                nc.scalar.dma_start(out_ap(o_d, b0), o_t[:])

            for it in range(n_iter):
                pend.append(stage_a(it))
                if len(pend) > PIPE:
                    stage_b(pend.pop(0))
            for s in pend:
                stage_b(s)
    nc.finalize()
    return nc


def _np_dt(mybir, dt):
    return mybir.dt.np(dt)


def _build_wstat():
    """Layout v3: partition dim = n (128 consecutive padded rows per tile,
    tiles overlap by 2), free dim = (batch, K) = 512. The whole 3-point
    stencil becomes ONE tridiagonal stationary W = (0.5, -1, 0.5):
        ps  = W @ x_tile + W @ p_tile          (2 matmuls, PSUM f32)
        o   = min(ps, 0) + x_tile              (single DVE STT)
    p rides in fp8 (only the PE reads it); x/o in bf16. Boundary rows
    use the BIG-sentinel x pad so out = x there.
    """
    import numpy as np
    import ml_dtypes
    import concourse.bacc as bacc
    import concourse.bass as bass
    import concourse.mybir as mybir
    from concourse.tile import TileContext

    f32 = mybir.dt.float32
    bf16 = mybir.dt.bfloat16
    AO = mybir.AluOpType
    p_dt = {"e3": mybir.dt.float8e3, "e4": mybir.dt.float8e4,
            "bf16": bf16}[P8]
    o_dt = mybir.dt.float8e3 if O8 else bf16
    p_np = _np_dt(mybir, p_dt)

    nc = bacc.Bacc()
    x_d = nc.dram_tensor("x", [NP, F], bf16, kind="ExternalInput")
    p_d = nc.dram_tensor("p", [NP, F], p_dt, kind="ExternalInput")
    o_d = nc.dram_tensor("o", [N, F], o_dt, kind="ExternalOutput")

    tri = (0.5 * np.eye(P, k=1) + 0.5 * np.eye(P, k=-1) - np.eye(P))
    wx_d = nc.inline_tensor(tri.astype(ml_dtypes.bfloat16), name="wx")
    wp_d = nc.inline_tensor(tri.astype(p_np), name="wp")

    with TileContext(nc) as tc:
        with (
            tc.tile_pool(name="const", bufs=1) as cpool,
            tc.tile_pool(name="io", bufs=BUFS) as pool,
            tc.tile_pool(name="ps", bufs=PSB, space="PSUM") as pspool,
        ):
            wx_t = cpool.tile([P, P], bf16, name="wx_t")
            wp_t = cpool.tile([P, P], p_dt, name="wp_t")
            nc.gpsimd.dma_start(wx_t[:], wx_d.ap())
            nc.gpsimd.dma_start(wp_t[:], wp_d.ap())

            pend = []
            TF = TSTRIDE * F  # DRAM elems between consecutive tile origins

            def stage_a(t0, nt):
                # nt consecutive full-stride tiles in one iteration: one
                # 3D-AP load per input, one MM pair per tile, one STT
                s = WS_LOAD[t0]
                x_t = pool.tile([P, nt * F], bf16, name="x_t")
                p_t = pool.tile([P, nt * F], p_dt, name="p_t")
                o_t = pool.tile([P, nt * F], o_dt, name="o_t")
                nc.sync.dma_start(
                    x_t[:], bass.AP(x_d, s * F, [[F, P], [TF, nt], [1, F]]))
                nc.scalar.dma_start(
                    p_t[:], bass.AP(p_d, s * F, [[F, P], [TF, nt], [1, F]]))
                ps = pspool.tile([P, nt * F], f32, name="ps")
                for j in range(nt):
                    c0 = j * F
                    nc.tensor.matmul(ps[:, c0:c0 + F], wx_t[:],
                                     x_t[:, c0:c0 + F],
                                     start=True, stop=False)
                    nc.tensor.matmul(ps[:, c0:c0 + F], wp_t[:],
                                     p_t[:, c0:c0 + F],
                                     start=False, stop=True)
                # o = min(w, 0) + x
                nc.vector.scalar_tensor_tensor(
                    o_t[:], ps[:], 0.0, x_t[:], AO.min, AO.add)
                return (t0, nt, o_t)

            def stage_b(state):
                t0, nt, o_t = state
                po, r0 = WS_PO[t0], WS_R0[t0]
                cnt = WS_CNT[t0]
                o3 = o_t.rearrange("p (j f) -> p j f", j=nt)
                nc.gpsimd.dma_start(
                    bass.AP(o_d, r0 * F,
                            [[F, cnt], [TSTRIDE * F, nt], [1, F]]),
                    o3[po:po + cnt, :, :])

            t0 = 0
            while t0 < WS_NT:
                if t0 < 65 - (65 % WT):
                    nt = WT
                else:
                    nt = 1  # trailing full tile (if odd) + ragged tile
                pend.append(stage_a(t0, nt))
                if len(pend) > PIPE:
                    stage_b(pend.pop(0))
                t0 += nt
            for s_ in pend:
                stage_b(s_)
    nc.finalize()
    return nc


def _shape_quant(p, f8):
    """Noise-shape p's quantization error toward low frequencies along n
    (the stencil (0.5,-1,0.5) is a high-pass and kills them): red-black
    coordinate descent on || h * (q - p) ||^2 over the fp8 grid."""
    import numpy as np

    p = np.ascontiguousarray(p, dtype=np.float32)
    q = p.astype(f8).astype(np.float32)
    n = p.shape[1]
    for _ in range(SHAPE_SWEEPS):
        for par in (0, 1):
            d = q - p
            dm2 = np.zeros_like(d); dm1 = np.zeros_like(d)
            dp1 = np.zeros_like(d); dp2 = np.zeros_like(d)
            dm2[:, 2:] = d[:, :-2]; dm1[:, 1:] = d[:, :-1]
            dp1[:, :-1] = d[:, 1:]; dp2[:, :-2] = d[:, 2:]
            a = 0.5 * dm2 - dm1
            b = 0.5 * (dm1 + dp1)
            c = 0.5 * dp2 - dp1
            dstar = (b - 0.5 * a - 0.5 * c) * (1.0 / 1.5)
            qn = (p + dstar).astype(f8).astype(np.float32)
            q[:, par::2] = qn[:, par::2]
    return q.astype(f8)


def _marshal_wstat(x, param):
    import numpy as np
    import ml_dtypes
    import concourse.mybir as mybir

    bf = ml_dtypes.bfloat16
    p_np = {"e3": ml_dtypes.float8_e3m4, "e4": ml_dtypes.float8_e4m3,
            "bf16": bf}[P8]

    x = np.ascontiguousarray(x, dtype=np.float32).reshape(NCORES, BPC, N, K)
    param = np.ascontiguousarray(param, dtype=np.float32)
    if P8 == "bf16":
        q = param.astype(bf).astype(p_np)
    else:
        q = _shape_quant(param.reshape(B, N, K), p_np)
    q = q.reshape(NCORES, BPC, N, K)

    in_maps = []
    for c in range(NCORES):
        xp = np.empty((NP, BPC, K), dtype=bf)
        xp[0] = BIG
        xp[NP - 1] = BIG
        xp[1:NP - 1] = x[c].transpose(1, 0, 2).astype(bf)
        pp = np.zeros((NP, BPC, K), dtype=p_np)
        pp[1:NP - 1] = q[c].transpose(1, 0, 2)
        in_maps.append({"x": xp.reshape(NP, F), "p": pp.reshape(NP, F)})
    return in_maps


def _build_pe_t(nc, bass, mybir, x_d, p_d, o_d, halo_ap, out_ap):
    """bf16 I/O + TensorEngine stencil accumulation. DVE's STT ops run at
    half rate (~115 G elem/s) and were the bottleneck of the all-DVE bf16
    build, so the shifted adds go to the (otherwise idle) PE as identity
    matmuls into PSUM:
        ps = x_up + p_up + x_dn + p_dn            (4 matmuls, stationary I)
        u  = 0.5*ps - p_ctr                       (DVE STT)
        o  = min(x_ctr, u)                        (DVE tensor_tensor)
    CONVEX_PE5=1 also folds -2*p_ctr into PSUM via a -2I stationary,
    leaving a single DVE op: o = min(0.5*ps, x_ctr).
    """
    import numpy as np
    import ml_dtypes
    from concourse.tile import TileContext

    bf16 = mybir.dt.bfloat16
    f32 = mybir.dt.float32
    AO = mybir.AluOpType
    FH = BPI * FHB
    FI = BPI * FIB
    n_iter = BPC // BPI
    PE5 = bool(int(os.environ.get("CONVEX_PE5", "0")))

    ident_d = nc.inline_tensor(
        np.eye(P, dtype=ml_dtypes.bfloat16), name="ident")
    if PE5:
        neg2_d = nc.inline_tensor(
            (-2.0 * np.eye(P)).astype(ml_dtypes.bfloat16), name="neg2")

    with TileContext(nc) as tc:
        with (
            tc.tile_pool(name="const", bufs=1) as cpool,
            tc.tile_pool(name="io", bufs=BUFS) as pool,
            tc.tile_pool(name="ps", bufs=PSB, space="PSUM") as pspool,
        ):
            # constants go out on the otherwise-idle SWDGE ring so the
            # first x/p loads are at the head of the HWDGE queues
            ident_t = cpool.tile([P, P], bf16, name="ident_t")
            nc.gpsimd.dma_start(ident_t[:], ident_d.ap())
            if PE5:
                neg2_t = cpool.tile([P, P], bf16, name="neg2_t")
                nc.gpsimd.dma_start(neg2_t[:], neg2_d.ap())

            pend = []

            def stage_a(it):
                b0 = it * BPI
                x_t = pool.tile([P, FH], bf16, name="x_t")
                p_t = pool.tile([P, FH], bf16, name="p_t")
                u_t = pool.tile([P, FI], bf16, name="u_t")

                nc.sync.dma_start(x_t[:], halo_ap(x_d, b0))
                nc.scalar.dma_start(p_t[:], halo_ap(p_d, b0))

                x3 = x_t.rearrange("p (q f) -> p q f", q=BPI)
                p3 = p_t.rearrange("p (q f) -> p q f", q=BPI)
                u3 = u_t.rearrange("p (q f) -> p q f", q=BPI)

                for q in range(BPI):
                    ps = pspool.tile([P, FIB], f32, name="ps")
                    for c0 in range(0, FIB, 512):
                        c1 = min(c0 + 512, FIB)
                        cps = ps[:, c0:c1]
                        nc.tensor.matmul(cps, ident_t[:],
                                         x3[:, q, c0:c1],
                                         start=True, stop=False)
                        nc.tensor.matmul(cps, ident_t[:],
                                         x3[:, q, 2 * K + c0:2 * K + c1],
                                         start=False, stop=False)
                        nc.tensor.matmul(cps, ident_t[:],
                                         p3[:, q, c0:c1],
                                         start=False, stop=False)
                        nc.tensor.matmul(cps, ident_t[:],
                                         p3[:, q, 2 * K + c0:2 * K + c1],
                                         start=False, stop=not PE5)
                        if PE5:
                            nc.tensor.matmul(cps, neg2_t[:],
                                             p3[:, q, K + c0:K + c1],
                                             start=False, stop=True)
                    uq = u3[:, q, :]
                    if PE5:
                        # o = min(0.5*ps, x_ctr) — single DVE op
                        nc.vector.scalar_tensor_tensor(
                            uq, ps[:], 0.5, x3[:, q, K:K + FIB],
                            AO.mult, AO.min)
                    else:
                        # u = 0.5*ps - p_ctr
                        nc.vector.scalar_tensor_tensor(
                            uq, ps[:], 0.5, p3[:, q, K:K + FIB],
                            AO.mult, AO.subtract)
                return (it, x_t, u_t)

            def stage_b(state):
                it, x_t, u_t = state
                b0 = it * BPI
                if PE5:
                    nc.gpsimd.dma_start(out_ap(o_d, b0), u_t[:])
                    return
                o_t = pool.tile([P, FI], bf16, name="o_t")
                x3 = x_t.rearrange("p (q f) -> p q f", q=BPI)
                o3 = o_t.rearrange("p (q f) -> p q f", q=BPI)
                u3 = u_t.rearrange("p (q f) -> p q f", q=BPI)
                nc.vector.tensor_tensor(o3[:], x3[:, :, K:K + FIB], u3[:],
                                        op=AO.min)
                nc.gpsimd.dma_start(out_ap(o_d, b0), o_t[:])

            for it in range(n_iter):
                pend.append(stage_a(it))
                if len(pend) > PIPE:
                    stage_b(pend.pop(0))
            for s in pend:
                stage_b(s)
    nc.finalize()
    return nc


def _build_bf16(nc, bass, mybir, x_d, p_d, o_d, halo_ap, out_ap):
    """All-bf16 I/O halves HBM traffic (the kernel is DMA-bound); the
    2e-2 rel-err gate leaves ~30x margin over bf16 rounding. DVE gets 2x
    throughput at 16-bit so all four element-wise passes stay well under
    the DMA floor:
        y  = x + p
        u1 = 0.5*y_up - p_ctr
        u  = 0.5*y_dn + u1       (= x_ctr - d, d the relu argument)
        o  = min(x_ctr, u)       (= x - relu(d))
    Loads split over the SP and ACT HWDGE rings; stores on SWDGE.
    """
    from concourse.tile import TileContext

    bf16 = mybir.dt.bfloat16
    AO = mybir.AluOpType
    FH = BPI * FHB
    FI = BPI * FIB
    n_iter = BPC // BPI

    with TileContext(nc) as tc:
        with tc.tile_pool(name="io", bufs=BUFS) as pool:
            pend = []

            def stage_a(it):
                b0 = it * BPI
                x_t = pool.tile([P, FH], bf16, name="x_t")
                p_t = pool.tile([P, FH], bf16, name="p_t")
                y_t = pool.tile([P, FH], bf16, name="y_t")
                u_t = pool.tile([P, FI], bf16, name="u_t")

                nc.sync.dma_start(x_t[:], halo_ap(x_d, b0))
                nc.scalar.dma_start(p_t[:], halo_ap(p_d, b0))
                nc.vector.tensor_tensor(y_t[:], x_t[:], p_t[:], op=AO.add)

                p3 = p_t.rearrange("p (q f) -> p q f", q=BPI)
                y3 = y_t.rearrange("p (q f) -> p q f", q=BPI)
                u3 = u_t.rearrange("p (q f) -> p q f", q=BPI)
                for q in range(BPI):
                    uq = u3[:, q, :]
                    nc.vector.scalar_tensor_tensor(
                        uq, y3[:, q, 0:FIB], 0.5, p3[:, q, K:K + FIB],
                        AO.mult, AO.subtract)
                    nc.vector.scalar_tensor_tensor(
                        uq, y3[:, q, 2 * K:2 * K + FIB], 0.5, uq,
                        AO.mult, AO.add)
                return (it, x_t, u_t)

            def stage_b(state):
                it, x_t, u_t = state
                b0 = it * BPI
                o_t = pool.tile([P, FI], bf16, name="o_t")
                x3 = x_t.rearrange("p (q f) -> p q f", q=BPI)
                o3 = o_t.rearrange("p (q f) -> p q f", q=BPI)
                u3 = u_t.rearrange("p (q f) -> p q f", q=BPI)
                nc.vector.tensor_tensor(o3[:], x3[:, :, K:K + FIB], u3[:],
                                        op=AO.min)
                nc.gpsimd.dma_start(out_ap(o_d, b0), o_t[:])

            for it in range(n_iter):
                pend.append(stage_a(it))
                if len(pend) > PIPE:
                    stage_b(pend.pop(0))
            for s in pend:
                stage_b(s)
    nc.finalize()
    return nc


def _build_pe_y(nc, bass, mybir, x_d, p_d, o_d, halo_ap, out_ap):
    """y = x + param on the TensorEngine (identity-matmul accumulate into
    PSUM), then per batch on DVE (each op reads at most one PSUM operand):
        u1 = 0.5*y_up - p_ctr
        u  = 0.5*y_dn + u1         (= x_ctr - d, with d the relu argument)
        o  = min(x_ctr, u)         (= x - relu(d))
    No relu, no PSUM->SBUF copy, no y-add on DVE. Loads split over the SP
    and ACT HWDGE rings; stores on SWDGE (GpSimd is otherwise idle).
    """
    import numpy as np
    from concourse.tile import TileContext

    f32 = mybir.dt.float32
    AO = mybir.AluOpType
    FH = BPI * FHB
    FI = BPI * FIB
    n_iter = BPC // BPI

    # bf16 identity is exact (1.0/0.0) and enables fast-weight-load;
    # f32 identity is the proven default
    ident_d = ident_bf_d = None
    if os.environ.get("CONVEX_IDENT_BF16"):
        import ml_dtypes
        ident_bf_d = nc.inline_tensor(
            np.eye(P, dtype=ml_dtypes.bfloat16), name="ident"
        )
    else:
        ident_d = nc.inline_tensor(np.eye(P, dtype=np.float32), name="ident")

    with TileContext(nc) as tc:
        with (
            tc.tile_pool(name="const", bufs=1) as cpool,
            tc.tile_pool(name="io", bufs=BUFS) as pool,
            tc.tile_pool(name="ps", bufs=2, space="PSUM") as pspool,
        ):
            if ident_d is not None:
                ident_t = cpool.tile([P, P], f32, name="ident_t")
                nc.sync.dma_start(ident_t[:], ident_d.ap())
            else:
                ident_t = cpool.tile([P, P], mybir.dt.bfloat16, name="ident_t")
                nc.sync.dma_start(ident_t[:], ident_bf_d.ap())

            pend = []

            def stage_a(it):
                b0 = it * BPI
                x_t = pool.tile([P, FH], f32, name="x_t")
                p_t = pool.tile([P, FH], f32, name="p_t")
                u_t = pool.tile([P, FI], f32, name="u_t")

                nc.sync.dma_start(x_t[:], halo_ap(x_d, b0))
                nc.scalar.dma_start(p_t[:], halo_ap(p_d, b0))

                x3 = x_t.rearrange("p (q f) -> p q f", q=BPI)
                p3 = p_t.rearrange("p (q f) -> p q f", q=BPI)
                u3 = u_t.rearrange("p (q f) -> p q f", q=BPI)

                if it < WARM:
                    # first iterations: y-add on DVE so nothing waits on a
                    # cold TensorEngine chain at startup
                    y_t = pool.tile([P, FH], f32, name="y_t")
                    nc.vector.tensor_tensor(y_t[:], x_t[:], p_t[:], op=AO.add)
                    y3 = y_t.rearrange("p (q f) -> p q f", q=BPI)
                    for q in range(BPI):
                        uq = u3[:, q, :]
                        nc.vector.scalar_tensor_tensor(
                            uq, y3[:, q, 0:FIB], 0.5, p3[:, q, K:K + FIB],
                            AO.mult, AO.subtract)
                        nc.vector.scalar_tensor_tensor(
                            uq, y3[:, q, 2 * K:2 * K + FIB], 0.5, uq,
                            AO.mult, AO.add)
                    return (it, x_t, u_t)

                for q in range(BPI):
                    ps = pspool.tile([P, FHB], f32, name="ps")
                    qo = q * FHB
                    # y = x + p, accumulated on the PE per <=512-col chunk
                    for c0 in range(0, FHB, 512):
                        c1 = min(c0 + 512, FHB)
                        nc.tensor.matmul(ps[:, c0:c1], ident_t[:],
                                         x_t[:, qo + c0:qo + c1],
                                         start=True, stop=False)
                        nc.tensor.matmul(ps[:, c0:c1], ident_t[:],
                                         p_t[:, qo + c0:qo + c1],
                                         start=False, stop=True)
                    uq = u3[:, q, :]
                    # u1 = 0.5*y_up - p_ctr ; u = 0.5*y_dn + u1
                    nc.vector.scalar_tensor_tensor(
                        uq, ps[:, 0:FIB], 0.5, p3[:, q, K:K + FIB],
                        AO.mult, AO.subtract)
                    nc.vector.scalar_tensor_tensor(
                        uq, ps[:, 2 * K:2 * K + FIB], 0.5, uq,
                        AO.mult, AO.add)
                return (it, x_t, u_t)

            def stage_b(state):
                it, x_t, u_t = state
                b0 = it * BPI
                o_t = pool.tile([P, FI], f32, name="o_t")
                x3 = x_t.rearrange("p (q f) -> p q f", q=BPI)
                o3 = o_t.rearrange("p (q f) -> p q f", q=BPI)
                u3 = u_t.rearrange("p (q f) -> p q f", q=BPI)
                # o = min(x_ctr, u) = x - relu(d)
                nc.vector.tensor_tensor(o3[:], x3[:, :, K:K + FIB], u3[:],
                                        op=AO.min)
                nc.gpsimd.dma_start(out_ap(o_d, b0), o_t[:])

            for it in range(n_iter):
                pend.append(stage_a(it))
                if len(pend) > PIPE:
                    stage_b(pend.pop(0))
            for s in pend:
                stage_b(s)
    nc.finalize()
    return nc


def _pad_inputs(x, param):
    # -> per-core padded slabs, shape [NCORES, BPC, NP, K]
    if STRATEGY in ("bf16", "pe_t"):
        import ml_dtypes
        io_np = ml_dtypes.bfloat16
    else:
        io_np = np.float32
    x = np.ascontiguousarray(x, dtype=np.float32).reshape(NCORES, BPC, N, K)
    param = np.ascontiguousarray(param, dtype=np.float32).reshape(NCORES, BPC, N, K)
    xp = np.empty((NCORES, BPC, NP, K), dtype=io_np)
    pp = np.empty((NCORES, BPC, NP, K), dtype=io_np)
    xp[:, :, 1:N + 1] = x.astype(io_np) if io_np is not np.float32 else x
    xp[:, :, 0] = BIG
    xp[:, :, N + 1] = BIG
    pp[:, :, 1:N + 1] = param.astype(io_np) if io_np is not np.float32 else param
    pp[:, :, 0] = 0.0
    pp[:, :, N + 1] = 0.0
    return xp, pp


def kernel(x: np.ndarray, param: np.ndarray) -> np.ndarray:
    global LAST_RESULTS
    from concourse.bass_utils import run_bass_kernel_spmd

    if "nc" not in _cache:
        _cache["nc"] = _build_wstat() if STRATEGY == "wstat" else _build_nc()
    nc = _cache["nc"]

    if STRATEGY == "wstat":
        in_maps = _marshal_wstat(x, param)
    else:
        xp, pp = _pad_inputs(x, param)
        in_maps = [{"x": xp[c], "p": pp[c]} for c in range(NCORES)]

    trace = bool(os.environ.get("BASS_TRACE"))
    res = run_bass_kernel_spmd(
        nc, in_maps, core_ids=list(range(NCORES)), trace=trace
    )
    LAST_RESULTS = res
    if STRATEGY == "wstat":
        out = np.stack([
            np.asarray(res.results[c]["o"])
            .astype(np.float32)
            .reshape(N, BPC, K)
            .transpose(1, 0, 2)
            for c in range(NCORES)
        ])
        return np.ascontiguousarray(out).reshape(B, N, K)
    out = np.concatenate([res.results[c]["o"] for c in range(NCORES)], axis=0)
    return out.reshape(B, N, K).astype(np.float32, copy=False)

